# revision 31
# baseline (speedup 1.0000x reference)
"""Tensor-parallel attention kernel for Trainium2 (8 NeuronCores).

Problem: B=1, L=2048, D=4096, H=32 q-heads, KV=8 kv-heads, HD=128,
partial rotary ROT=64, causal additive mask, o-projection.

Sharding: TP-8 over heads. Core c owns q-heads 4c..4c+3 and kv-head c
(column shard of w_qkv), plus the matching row shard of w_o. Each core
computes a full [L, D] partial of the output; the host sums the 8
partials (the cross-core reduction of the row-sharded o-projection).

All on-chip data is bf16 (PSUM accumulation stays f32): halves HBM
traffic vs f32 and lets the PE use FWL weight loads. rel-err budget is
2e-2; bf16 end-to-end measures ~4e-3.

Everything runs in "transposed" orientation so every matmul contracts
over the partition dim with zero on-chip activation transposes:
  qkvT[col, L] = w_qkv.T @ x.T          (w stationary, xT streamed)
  rope:  qT' = qT * cosE + (P @ qT) * sinE   (P = rotate-half matrix on PE)
  ST[k, q]   = kT_tile.T @ qT            (one matmul per k-tile, K=HD=128)
  PT         = exp(ST + tri)             (exp batched over k-tile pairs)
  den[*, q]  = ones.T @ PT               (ones-matmul, accumulated over k)
  oT[d, q]   = V_tile.T @ PT             (V from a one-time PE transpose of vT)
  out[l, e]  = (oT/den).T @ w_o_shard    (partial; summed across cores on host)

Causality is exploited at 128-row k-strip granularity: strips fully
below the diagonal skip the mask entirely; diagonal strips compute only
the valid q-subrange and add a single shared [128,128] triangle mask.
"""

import sys

for _p in ("/opt/trn_rl_repo", "/root/.axon_site/_ro/trn_rl_repo"):
    if _p not in sys.path:
        sys.path.append(_p)

import numpy as np
import ml_dtypes

B, L, D = 1, 2048, 4096
H, KV, HD = 32, 8, 128
ROT = 64
SCALE = HD ** -0.5
NEG = -1e9
NCORES = 8
HPC = H // NCORES          # q-heads per core (4)
CPC = HPC * HD + 2 * HD    # w_qkv columns per core (768)
NDT = D // 128             # contraction tiles over D (32)
NKT = L // 128             # k tiles (16)
NJQ = L // 512             # 512-wide q blocks (4)
XBLK = 512                 # L-block width in the qkv phase
NLB = L // XBLK            # 4

NPBF16 = ml_dtypes.bfloat16

_cache = {}


def _build(causal: bool):
    import concourse.mybir as mybir
    import concourse.tile as tile
    from concourse import bacc

    F32 = mybir.dt.float32
    BF16 = mybir.dt.bfloat16
    EXP = mybir.ActivationFunctionType.Exp

    nc = bacc.Bacc("TRN2", target_bir_lowering=False, debug=False)

    # all big operands are pre-tiled on the host so every DMA line is a
    # long contiguous run (8-32KB): [partition, ...free] layouts
    xt_r = nc.dram_tensor("xt", [NLB, 128, NDT, XBLK], BF16,
                          kind="ExternalInput").ap()
    # w_qkv as two column-half planes so a (quarter, half) DMA piece has
    # 6KB contiguous lines
    wqkv_r = nc.dram_tensor("wqkv", [2, 128, NDT, CPC // 2], BF16,
                            kind="ExternalInput").ap()
    wo_r = nc.dram_tensor("wo", [128, HPC, D], BF16, kind="ExternalInput").ap()
    tabs_t = nc.dram_tensor("tabs", [NLB, 128, 4, XBLK], F32,
                            kind="ExternalInput").ap()
    # sign-folded bf16 sin tables for the DMA-shift rope (rows 0:32 negated)
    tabs16_t = nc.dram_tensor("tabs16", [NLB, 128, 2, XBLK], BF16,
                              kind="ExternalInput").ap()
    consts = nc.dram_tensor("consts", [128, 512], BF16,
                            kind="ExternalInput").ap()
    if not causal:
        mask_t = nc.dram_tensor("mask_t", [L, L], F32, kind="ExternalInput").ap()
    out_p = nc.dram_tensor("out_p", [L, D], BF16, kind="ExternalOutput").ap()

    with tile.TileContext(nc) as tc:
        with tc.tile_pool(name="persist", bufs=1) as persist:
            kt_sb = persist.tile([128, L], BF16, tag="kt")
            v_sb = persist.tile([128, NKT, 128], BF16, tag="v")
            qt_sb = persist.tile([128, HPC, L], BF16, tag="qt")
            wo_sb = persist.tile([128, HPC, D], BF16, tag="wo")
            cst = persist.tile([128, 512], BF16, tag="cst")
            nc.sync.dma_start(out=cst, in_=consts)
            ident = cst[:, 0:128]
            ones = cst[:, 128:256]
            pmat_t = cst[:, 256:384]
            tri01 = cst[:, 384:512]

            # ---------------- Phase 1: qkv projection + rope ----------------
            with tc.tile_pool(name="wq", bufs=1) as wqp, \
                 tc.tile_pool(name="xb", bufs=2) as xbp, \
                 tc.tile_pool(name="tabs", bufs=2) as tabs, \
                 tc.tile_pool(name="stage", bufs=3) as stage, \
                 tc.tile_pool(name="vtmp", bufs=2) as vtmp, \
                 tc.tile_pool(name="ps1", bufs=6, space="PSUM") as ps1:
                wq_sb = wqp.tile([128, 2, NDT, CPC // 2], BF16)

                # weight DMA split into (dti-quarter x column-half-plane)
                # pieces, interleaved with lb0's x quarters across the sync
                # and gpsimd queues in exact consumption order (scalar's DGE
                # path is ~6x slower per DMA instruction — it only gets the
                # rope tables)
                def wq_piece(eng, e8, colh):
                    qs_ = slice(4 * e8, 4 * e8 + 4)
                    eng.dma_start(out=wq_sb[:, colh, qs_, :],
                                  in_=wqkv_r[colh][:, qs_, :])

                def wq_lhs(ct, dti):
                    c0 = (ct % 3) * 128
                    return wq_sb[:, ct // 3, dti, c0:c0 + 128]

                rope_defer = []

                def rope_ct(lb, ct, acc, tb, tb16):
                    """Consume psum acc for column-group ct of L-block lb.

                    Rotate-half runs as two SBUF partition-shift DMAs on the
                    scalar queue with the sign folded into the bf16 sin table
                    — no Tensor-engine work in the rope at all. Only the ops
                    that release the PSUM acc issue now; the shift-dependent
                    ops are deferred so they never block the Vector queue
                    ahead of the next block's acc releases.
                    """
                    ls = slice(lb * XBLK, (lb + 1) * XBLK)
                    if ct == 5:
                        # v: stage, then XBAR-transpose this block's k-tiles
                        vt_sb = vtmp.tile([128, XBLK], BF16, tag="vt")
                        nc.scalar.copy(out=vt_sb, in_=acc)
                        for kk in range(XBLK // 128):
                            i = lb * (XBLK // 128) + kk
                            nc.scalar.dma_start_transpose(
                                out=v_sb[:, i, :],
                                in_=vt_sb[:, kk * 128:(kk + 1) * 128])
                        return
                    # rope for q (ct 0..3, scaled tables) and k (ct 4)
                    ti = 0 if ct < 4 else 1
                    s_sb = stage.tile([128, XBLK], BF16, tag="s_sb")
                    nc.scalar.copy(out=s_sb, in_=acc)
                    # all 5 xsh tiles of a block stay live until the deferred
                    # flush at the next block's top — needs its own deep ring
                    xsh = stage.tile([128, XBLK], BF16, tag="xsh", bufs=6)
                    nc.scalar.dma_start(out=xsh[0:32, :], in_=s_sb[32:64, :])
                    nc.scalar.dma_start(out=xsh[32:64, :], in_=s_sb[0:32, :])
                    dst = qt_sb[:, ct, ls] if ct < 4 else kt_sb[:, ls]
                    nc.vector.tensor_mul(dst, acc, tb[:, ti, :])
                    rope_defer.append((xsh, dst, ti, tb16))

                def rope_flush():
                    for xsh, dst, ti, tb16 in rope_defer:
                        m2 = stage.tile([128, XBLK], BF16, tag="m2",
                                        name="m2d")
                        nc.vector.tensor_mul(m2[0:64, :], xsh[0:64, :],
                                             tb16[0:64, ti, :])
                        nc.vector.tensor_add(dst[0:64, :], dst[0:64, :],
                                             m2[0:64, :])
                    rope_defer.clear()

                for lb in range(NLB):
                    rope_flush()
                    xblk = xbp.tile([128, NDT, XBLK], BF16, tag="xblk")
                    if lb == 0:
                        def xq_piece(eng, e8):
                            qs_ = slice(4 * e8, 4 * e8 + 4)
                            eng.dma_start(out=xblk[:, qs_, :],
                                          in_=xt_r[lb][:, qs_, :])

                        # finest-grained (4-dti) pieces, strictly in PE
                        # consumption order, alternating across the two fast
                        # queues so both stream in parallel from t=0
                        qq = (nc.sync, nc.gpsimd)
                        for e8 in range(8):
                            xq_piece(qq[e8 % 2], e8)
                            wq_piece(qq[(e8 + 1) % 2], e8, 0)
                            wq_piece(qq[e8 % 2], e8, 1)
                    else:
                        # lb>0 x blocks split across both queues; wo rides
                        # last on gpsimd (needed only in phase 3)
                        nc.sync.dma_start(out=xblk[:, 0:16, :],
                                          in_=xt_r[lb][:, 0:16, :])
                        nc.gpsimd.dma_start(out=xblk[:, 16:32, :],
                                            in_=xt_r[lb][:, 16:32, :])
                        if lb == NLB - 1:
                            nc.gpsimd.dma_start(out=wo_sb, in_=wo_r)
                    tb = tabs.tile([128, 4, XBLK], F32, tag="tb")
                    nc.scalar.dma_start(out=tb, in_=tabs_t[lb])
                    tb16 = tabs.tile([128, 2, XBLK], BF16, tag="tb16")
                    nc.scalar.dma_start(out=tb16, in_=tabs16_t[lb])
                    if lb == 0:
                        # single pass, all 6 column-groups accumulating in
                        # parallel (6 PSUM banks), dti-eighth inner: the DMA
                        # demand stays flat and the first matmuls only wait
                        # on the first ~0.8MB of pieces
                        accs = {}
                        for ct in range(6):
                            accs[ct] = ps1.tile(
                                [128, XBLK], F32, tag="acc",
                                name=f"acc_l0_{ct}")
                        for e8 in range(8):
                            for ct in range(6):
                                for dti in range(4 * e8, 4 * e8 + 4):
                                    nc.tensor.matmul(
                                        out=accs[ct],
                                        lhsT=wq_lhs(ct, dti),
                                        rhs=xblk[:, dti, :],
                                        start=(dti == 0),
                                        stop=(dti == NDT - 1))
                        for ct in range(6):
                            rope_ct(lb, ct, accs[ct], tb, tb16)
                    else:
                        for ct in range(6):
                            acc = ps1.tile([128, XBLK], F32, tag="acc")
                            for dti in range(NDT):
                                nc.tensor.matmul(
                                    out=acc,
                                    lhsT=wq_lhs(ct, dti),
                                    rhs=xblk[:, dti, :],
                                    start=(dti == 0), stop=(dti == NDT - 1))
                            rope_ct(lb, ct, acc, tb, tb16)
                            if lb == NLB - 1:
                                # last block: nothing left to protect on the
                                # Vector queue, and phase 2's first matmuls
                                # wait (coarse-grained) on qt/kt writes —
                                # finish each rope immediately
                                rope_flush()
                rope_flush()

            # ---------------- Phases 2+3 ----------------
            late_cm = tc.tile_pool(name="late", bufs=1)
            late = late_cm.__enter__()
            otn_sb = late.tile([128, HPC, L], BF16, tag="otn")

            # ---------------- Phase 2: attention ----------------
            with tc.tile_pool(name="mb", bufs=2) as mbp, \
                 tc.tile_pool(name="pt", bufs=4) as ptp, \
                 tc.tile_pool(name="pts", bufs=2) as ptsp, \
                 tc.tile_pool(name="rdp", bufs=2) as rdp, \
                 tc.tile_pool(name="ps_st", bufs=2, space="PSUM") as ps_st, \
                 tc.tile_pool(name="ps_acc", bufs=2, space="PSUM") as ps_acc:
                # pend carries one pair of exp'd k-strips across loop
                # iterations (including across (jq,h) group boundaries), so
                # the den/ot matmuls of a pair always issue after the NEXT
                # pair's S matmuls — the exp latency hides under PE work
                pend = [None]

                def flush():
                    if pend[0] is None:
                        return
                    (pt2, specs, ptsum, den, ot, first_, last_, h_, qs_) = \
                        pend[0]
                    if ptsum is not None:
                        # full pair: one den matmul on the pre-summed pt,
                        # two ot matmuls
                        nc.tensor.matmul(
                            out=den, lhsT=ones[:, 0:128], rhs=ptsum,
                            start=first_, stop=last_,
                            skip_group_check=causal)
                        for i2, (i, ds, w) in enumerate(specs):
                            nc.tensor.matmul(
                                out=ot, lhsT=v_sb[:, i, :],
                                rhs=pt2[:, i2, :],
                                start=first_ and i2 == 0,
                                stop=last_ and i2 == 1,
                                skip_group_check=causal)
                    else:
                        for i2, (i, ds, w) in enumerate(specs):
                            st_ = first_ and i2 == 0
                            sp_ = last_ and i2 == 1
                            nc.tensor.matmul(
                                out=den[:, ds:ds + w],
                                lhsT=ones[:, 0:128],
                                rhs=pt2[:, i2, 0:w],
                                start=st_, stop=sp_,
                                skip_group_check=causal)
                            nc.tensor.matmul(
                                out=ot[:, ds:ds + w],
                                lhsT=v_sb[:, i, :],
                                rhs=pt2[:, i2, 0:w],
                                start=st_, stop=sp_,
                                skip_group_check=causal)
                    if last_:
                        rd = rdp.tile([128, 512], F32, tag="rd")
                        nc.vector.reciprocal_approx_fast(out=rd, in_=den)
                        nc.vector.tensor_mul(otn_sb[:, h_, qs_], ot, rd)
                    pend[0] = None

                for jq in range(NJQ):
                    qs = slice(jq * 512, (jq + 1) * 512)
                    if not causal:
                        mblk = mbp.tile([128, NKT, 512], F32, tag="mblk")
                        nc.scalar.dma_start(
                            out=mblk,
                            in_=mask_t[:, qs].rearrange("(kt p) q -> p kt q",
                                                        p=128))
                    for h in range(HPC):
                        den = ps_acc.tile([128, 512], F32, tag="den")
                        ot = ps_acc.tile([128, 512], F32, tag="ot")
                        nfull = 4 * jq if causal else NKT
                        ndiag = 4 if causal else 0
                        npairs = (nfull + ndiag) // 2
                        for m in range(npairs):
                            st2 = ps_st.tile([128, 2, 512], F32, tag="st2")
                            pt2 = ptp.tile([128, 2, 512], BF16, tag="pt2")
                            specs = []
                            all_full = 2 * m + 1 < nfull
                            for i2 in range(2):
                                i = 2 * m + i2
                                if i < nfull:
                                    ds, w = 0, 512
                                    qsl = qs
                                else:
                                    d = i - nfull
                                    ds, w = 128 * d, 512 - 128 * d
                                    qsl = slice(jq * 512 + 128 * d,
                                                (jq + 1) * 512)
                                nc.tensor.matmul(
                                    out=st2[:, i2, 0:w],
                                    lhsT=kt_sb[:, i * 128:(i + 1) * 128],
                                    rhs=qt_sb[:, h, qsl],
                                    start=True, stop=True)
                                if not causal:
                                    nc.vector.tensor_add(
                                        st2[:, i2, :], st2[:, i2, :],
                                        mblk[:, i, :])
                                specs.append((i, ds, w))
                            ptsum = None
                            if all_full or not causal:
                                nc.scalar.activation(pt2, st2, EXP)
                                ptsum = ptsp.tile([128, 512], BF16,
                                                  tag="ptsum")
                                nc.vector.tensor_add(
                                    ptsum, pt2[:, 0, :], pt2[:, 1, :])
                            else:
                                # diagonal strips: exp only the valid width,
                                # then zero the masked triangle in SBUF (keeps
                                # the Vector engine off the PSUM WAR chain)
                                for i2, (i, ds, w) in enumerate(specs):
                                    nc.scalar.activation(
                                        pt2[:, i2, 0:w], st2[:, i2, 0:w], EXP)
                                    nc.vector.tensor_mul(
                                        pt2[:, i2, 0:128], pt2[:, i2, 0:128],
                                        tri01)
                            flush()
                            pend[0] = (pt2, specs, ptsum, den, ot,
                                       m == 0, m == npairs - 1, h, qs)
                flush()

            # ---------------- Phase 3: o-projection ----------------
            with tc.tile_pool(name="ost", bufs=2) as ostp, \
                 tc.tile_pool(name="ps3", bufs=6, space="PSUM") as ps3:
                for lt in range(L // 128):
                    ost = ostp.tile([128, D], BF16, tag="ost")
                    for et in range(D // 512):
                        es = slice(et * 512, (et + 1) * 512)
                        acc = ps3.tile([128, 512], F32, tag="acc3")
                        for h in range(HPC):
                            nc.tensor.matmul(
                                out=acc,
                                lhsT=otn_sb[:, h, lt * 128:(lt + 1) * 128],
                                rhs=wo_sb[:, h, es],
                                start=(h == 0), stop=(h == HPC - 1))
                        if et % 2 == 0:
                            nc.vector.tensor_copy(ost[:, es], acc)
                        else:
                            nc.scalar.copy(out=ost[:, es], in_=acc)
                        if et == 3:
                            nc.sync.dma_start(
                                out=out_p[lt * 128:(lt + 1) * 128, 0:2048],
                                in_=ost[:, 0:2048])
                    nc.sync.dma_start(
                        out=out_p[lt * 128:(lt + 1) * 128, 2048:4096],
                        in_=ost[:, 2048:4096])

            late_cm.__exit__(None, None, None)

    nc.compile()
    return nc


def _host_inputs(x, attention_mask, cos, sin, w_qkv, w_o, causal):
    """Build the 8 per-core input maps (bf16 data, host pre-tiled so every
    DMA line is a long contiguous run)."""
    q_pos = H * HD
    kv_pos = q_pos + KV * HD

    # x pre-tiled: [NLB, 128(p), NDT(dt), XBLK(j)] with
    # value = x[lb*XBLK + j, dt*128 + p]
    xt = np.ascontiguousarray(
        x[0].reshape(NLB, XBLK, NDT, 128).transpose(0, 3, 2, 1)
    ).astype(NPBF16)

    # rope tables pre-tiled: [NLB, 128, 4, XBLK], slots =
    # [cos_q(scale folded), cos_k, sin_q(scaled), sin_k]
    cos_t = cos.T.astype(np.float32)                      # [ROT, L]
    sin_t = sin.T.astype(np.float32)
    tab = np.zeros((4, 128, L), np.float32)
    tab[0, :ROT] = cos_t * SCALE
    tab[0, ROT:] = SCALE
    tab[1, :ROT] = cos_t
    tab[1, ROT:] = 1.0
    tab[2, :ROT] = sin_t * SCALE
    tab[3, :ROT] = sin_t
    tabs = np.ascontiguousarray(
        tab.reshape(4, 128, NLB, XBLK).transpose(2, 1, 0, 3))
    # bf16 sin tables with the rotate-half sign folded in: rows 0:32 negated
    t16 = tab[2:4].copy()
    t16[:, 0:32] = -t16[:, 0:32]
    t16[:, 64:] = 0.0
    tabs16 = np.ascontiguousarray(
        t16.reshape(2, 128, NLB, XBLK).transpose(2, 1, 0, 3)).astype(NPBF16)

    # consts [128, 512] = [identity | ones | pmat_t | tri01] (bf16)
    pmat = np.zeros((128, 128), np.float32)
    for dp in range(32):
        pmat[dp, dp + 32] = -1.0
    for dp in range(32, 64):
        pmat[dp, dp - 32] = 1.0
    # 0/1 triangle applied to exp(S) on diagonal blocks, [k, q]: keep q >= k
    kk, qq = np.meshgrid(np.arange(128), np.arange(128), indexing="ij")
    tri01 = (qq >= kk).astype(np.float32)
    consts = np.concatenate(
        [np.eye(128, dtype=np.float32), np.ones((128, 128), np.float32),
         pmat.T, tri01], axis=1).astype(NPBF16)

    mask2d = np.ascontiguousarray(attention_mask[0, 0])   # [L(q), L(k)]
    if causal:
        mask_t_full = None
    else:
        mask_t_full = np.ascontiguousarray(mask2d.T.astype(np.float32))

    in_maps = []
    for c in range(NCORES):
        cols = []
        for j in range(HPC):
            h = c * HPC + j
            cols.append(w_qkv[:, h * HD:(h + 1) * HD])
        cols.append(w_qkv[:, q_pos + c * HD:q_pos + (c + 1) * HD])
        cols.append(w_qkv[:, kv_pos + c * HD:kv_pos + (c + 1) * HD])
        wqkv_c = np.concatenate(cols, axis=1)                        # [D, 768]
        # pre-tile into column-half planes: [2(colh), 128(p), NDT(dt), 384]
        wqkv_c = np.ascontiguousarray(
            wqkv_c.reshape(NDT, 128, 2, CPC // 2)
            .transpose(2, 1, 0, 3)).astype(NPBF16)
        wo_c = w_o[c * HPC * HD:(c + 1) * HPC * HD, :]               # [512, D]
        # pre-tile: [128(p), HPC(h), D(e)]
        wo_c = np.ascontiguousarray(
            wo_c.reshape(HPC, 128, D).transpose(1, 0, 2)).astype(NPBF16)
        m = {"xt": xt, "wqkv": wqkv_c, "wo": wo_c,
             "tabs": tabs, "tabs16": tabs16, "consts": consts}
        if not causal:
            m["mask_t"] = mask_t_full
        in_maps.append(m)
    return in_maps


def _is_causal(mask2d):
    expected = np.where(
        np.tril(np.ones((L, L), dtype=bool)), np.float32(0.0), np.float32(NEG))
    return mask2d.shape == (L, L) and np.array_equal(mask2d, expected)


def kernel(x, attention_mask, cos, sin, w_qkv, w_o, _trace=False):
    from concourse.bass_utils import run_bass_kernel_spmd

    x = np.asarray(x, dtype=np.float32)
    attention_mask = np.asarray(attention_mask, dtype=np.float32)
    cos = np.asarray(cos, dtype=np.float32)
    sin = np.asarray(sin, dtype=np.float32)
    w_qkv = np.asarray(w_qkv, dtype=np.float32)
    w_o = np.asarray(w_o, dtype=np.float32)

    causal = _is_causal(attention_mask[0, 0])
    if causal not in _cache:
        _cache[causal] = _build(causal)
    nc = _cache[causal]

    in_maps = _host_inputs(x, attention_mask, cos, sin, w_qkv, w_o, causal)
    try:
        res = run_bass_kernel_spmd(nc, in_maps, list(range(NCORES)), trace=_trace)
    except Exception:
        # transient device errors (e.g. NRT_EXEC_UNIT_UNRECOVERABLE) usually
        # clear on retry
        res = run_bass_kernel_spmd(nc, in_maps, list(range(NCORES)), trace=_trace)
    out = np.zeros((L, D), np.float64)
    for c in range(NCORES):
        out += res.results[c]["out_p"].astype(np.float64)
    if _trace:
        kernel._last_exec_time_ns = res.exec_time_ns
    return out.astype(np.float32).reshape(B, L, D)
